# revision 13
# baseline (speedup 1.0000x reference)
"""CosineGatedAttentionUnit Trainium2 kernel (8 NeuronCores, SPMD), v2.

Sharding: core c -> batch b = c//4, heads (2*(c%4), 2*(c%4)+1).
Each core computes its two heads' attention output, multiplies by its gate
slice, contracts against its Wo row-slice, and returns a partial [N, C]
result; the host sums the 4 partials per batch and adds bo.

v2 design notes (vs v1):
  - LayerNorm + transpose + bf16 cast happen on the host (mirrors the
    host-side exp(pos_bias) prep v1 already did).  The device receives
    xnT [C, N] bf16 ready to be the moving operand of every projection.
    This removes the LN->transpose serial ramp that idled the PE ~45us
    and kept it at a low DVFS p-state.
  - Attention works on i-chunks of 1024 (PSUM tile [128,1024] spanning
    2 banks, filled by two 512-wide matmuls).  Halves ACT/DVE
    instruction count vs v1's 512-wide tiles.
  - The softmax row-sum no longer uses PE ones-matmuls per j (27us of
    PE in v1).  exp values are pair-summed in bf16 and accumulated into
    two fp32 accumulators, alternating DVE / Pool(gpsimd) so neither
    engine becomes the bottleneck.  One tiny f32r reduce matmul per
    (h, ic) collapses the partition axis.
  - 1/rowsum is broadcast to 128 partitions with a partition-stride-0
    SBUF->SBUF DMA instead of a PE matmul + PSUM round trip.
  - out2T = (attn@v) * gate * (1/rowsum) is restructured so oa (PSUM) is
    freed immediately after the j loop (og = oa*gate on DVE), letting the
    next (h, ic) iteration start accumulating without waiting on the
    softmax denominator chain; the rest of the epilogue overlaps the next
    iteration's j loop (emitted at its j==0/j==1).
  - PSUM budget (8 banks): dots ring 2x[128,1024] (4) + oa0/oa1 (4);
    the reduce row borrows the oa0 ring slot between its last read and
    the next iteration's first accumulate.
  - All-zero biases (as produced by setup_inputs) skip the bias ops;
    nonzero biases take the v1 paths, selected at build time.
"""

import math

import ml_dtypes
import numpy as np

import concourse.bass as bass
import concourse.mybir as mybir
import concourse.tile as tile
from concourse.bass_utils import run_bass_kernel_spmd

# ---- problem constants -------------------------------------------------
B, N, C, H, D, E = 2, 2048, 1024, 8, 64, 2
DV = C * E // H  # 256
NT = N // 128    # 16 token tiles
CCN = C // 128   # 8 contraction chunks
EPS = 1e-5

F32 = mybir.dt.float32
F32R = mybir.dt.float32r
BF16 = mybir.dt.bfloat16
OP = mybir.AluOpType
AF = mybir.ActivationFunctionType


# ---- walrus workaround: 1 sync wait per instruction --------------------
WAIT_LIMIT = 1


def split_excess_waits(nc: bass.Bass, limit: int = WAIT_LIMIT):
    n_split = 0
    for f in nc.m.functions:
        for bb in f.blocks:
            out = []
            for inst in bb.instructions:
                si = inst.sync_info
                if si is not None and len(si.on_wait) > limit:
                    waits = list(si.on_wait)
                    extra, keep = waits[:-limit], waits[-limit:]
                    k = 0
                    while extra:
                        grp, extra = extra[:limit], extra[limit:]
                        nop = mybir.InstNoOp(
                            name=f"{inst.name}-ws{k}",
                            engine=inst.engine,
                            sync_info=mybir.SyncInfo(on_wait=grp, on_update=[]),
                        )
                        out.append(nop)
                        k += 1
                    inst.sync_info = mybir.SyncInfo(
                        on_wait=keep, on_update=list(si.on_update))
                    n_split += 1
                out.append(inst)
            bb.instructions = out
    return n_split


# ---- device program ----------------------------------------------------
def build_program(temperature: float, has_qk_bias: bool = False,
                  has_vg_bias: bool = False,
                  split_waits: bool = True) -> bass.Bass:
    nc = bass.Bass("TRN2", target_bir_lowering=False, debug=False,
                   num_devices=8)

    xnt_d = nc.dram_tensor("xnt", [C, N], BF16, kind="ExternalInput")
    wq_d = nc.dram_tensor("wq", [C, 128], BF16, kind="ExternalInput")
    wk_d = nc.dram_tensor("wk", [C, 128], BF16, kind="ExternalInput")
    wv_d = nc.dram_tensor("wv", [C, 512], BF16, kind="ExternalInput")
    wg_d = nc.dram_tensor("wg", [C, 512], BF16, kind="ExternalInput")
    wo_d = nc.dram_tensor("wo", [512, C], BF16, kind="ExternalInput")
    pbt_d = nc.dram_tensor("pbt", [2, N, N], BF16, kind="ExternalInput")
    sels_d = nc.dram_tensor("sel_stats", [128, 2], F32R, kind="ExternalInput")
    selb_d = nc.dram_tensor("sel_bcast", [2, 128], F32R, kind="ExternalInput")
    onessq_d = nc.dram_tensor("onessq", [128, 128], F32R, kind="ExternalInput")

    if has_qk_bias:
        bqk_d = nc.dram_tensor("bqk", [128, 2], F32, kind="ExternalInput")
    if has_vg_bias:
        bv_d = nc.dram_tensor("bv", [512], F32, kind="ExternalInput")
        bg_d = nc.dram_tensor("bg", [128, 4], F32, kind="ExternalInput")
    out_d = nc.dram_tensor("out", [N, C], F32, kind="ExternalOutput")

    out_ap = out_d.ap()
    lnT = math.log(temperature)

    with tile.TileContext(nc, pool_alloc_mode="queue") as tc:
        with tc.tile_pool(name="consts", bufs=1) as consts:
            sel_stats = consts.tile([128, 2], F32R, name="sel_stats")
            nc.sync.dma_start(sel_stats, sels_d.ap())
            sel_bcast = consts.tile([2, 128], F32R, name="sel_bcast")
            nc.sync.dma_start(sel_bcast, selb_d.ap())
            ones_sq = consts.tile([128, 128], F32R, name="ones_sq")
            nc.sync.dma_start(ones_sq, onessq_d.ap())
            lnT_t = consts.tile([2, 1], F32, name="lnT_t")
            nc.vector.memset(lnT_t, lnT)
            zero2_t = consts.tile([2, 1], F32, name="zero2_t")
            nc.vector.memset(zero2_t, 0.0)
            if has_qk_bias:
                bqk_sb = consts.tile([128, 2], F32, name="bqk_sb")
                nc.sync.dma_start(bqk_sb, bqk_d.ap())
            if has_vg_bias:
                bv_sb = consts.tile([128, 512], F32, name="bv_sb")
                nc.sync.dma_start(bv_sb, bass.AP(bv_d, 0, [[0, 128], [1, 512]]))
                bg_sb = consts.tile([128, 4], F32, name="bg_sb")
                nc.sync.dma_start(bg_sb, bg_d.ap())

            with tc.tile_pool(name="resid", bufs=1) as resid:
                qst = resid.tile([128, N], BF16, name="qst")
                kst = resid.tile([128, N], BF16, name="kst")
                v_sb = [
                    resid.tile([128, 512], BF16, name=f"v_{tt}", tag=f"v_{tt}")
                    for tt in range(NT)
                ]
                gateT = [
                    resid.tile([128, N], BF16, name=f"gt_{q}", tag=f"gt_{q}")
                    for q in range(4)
                ]
                out2T = [
                    resid.tile([128, N], BF16, name=f"o2_{q}", tag=f"o2_{q}")
                    for q in range(4)
                ]
                wo_sb = [
                    resid.tile([128, C], BF16, name=f"wo_{q}", tag=f"wo_{q}")
                    for q in range(4)
                ]

                # ------------- phase P: projections ----------------------
                with tc.tile_pool(name="xw", bufs=1) as xw, \
                     tc.tile_pool(name="pp", bufs=1, space="PSUM") as pp:
                    xnT = []
                    for cc in range(CCN):
                        t = xw.tile([128, N], BF16, name=f"xnT_{cc}",
                                    tag=f"xnT_{cc}")
                        nc.sync.dma_start(
                            t, xnt_d.ap()[cc * 128:(cc + 1) * 128, :])
                        xnT.append(t)
                    w_sb = {}
                    for wname, wd in (("q", wq_d), ("k", wk_d)):
                        for cc in range(CCN):
                            wt = xw.tile([128, 128], BF16,
                                         name=f"w{wname}_{cc}",
                                         tag=f"w{wname}_{cc}")
                            nc.sync.dma_start(
                                wt, wd.ap()[cc * 128:(cc + 1) * 128, :])
                            w_sb[(wname, cc)] = wt
                    wv_sb, wg_sb = [], []
                    for lst, wd, nm in ((wv_sb, wv_d, "wv"), (wg_sb, wg_d, "wg")):
                        for cc in range(CCN):
                            wt = xw.tile([128, 512], BF16, name=f"{nm}_{cc}",
                                         tag=f"{nm}_{cc}")
                            nc.sync.dma_start(
                                wt, wd.ap()[cc * 128:(cc + 1) * 128, :])
                            lst.append(wt)
                    # wo DMAs late in SP queue order (used last)
                    for q in range(4):
                        nc.sync.dma_start(
                            wo_sb[q], wo_d.ap()[q * 128:(q + 1) * 128, :])

                    # Q/K projections, silu, l2norm * T scale
                    for wi, (wname, dst) in enumerate((("q", qst), ("k", kst))):
                        pr = [
                            pp.tile([128, 512], F32, name=f"pr{i}",
                                    tag=f"pr{i}", bufs=1)
                            for i in range(4)
                        ]
                        for cc in range(CCN):
                            for i in range(4):
                                nc.tensor.matmul(
                                    pr[i],
                                    lhsT=w_sb[(wname, cc)],
                                    rhs=xnT[cc][:, i * 512:(i + 1) * 512],
                                    start=(cc == 0), stop=(cc == CCN - 1),
                                )
                        silu = xw.tile([128, N], F32, name="silu",
                                       tag="silu", bufs=2)
                        for i in range(4):
                            isl = slice(i * 512, (i + 1) * 512)
                            sig = xw.tile([128, 512], F32, name="sig",
                                          tag="sig", bufs=2)
                            if has_qk_bias:
                                nc.scalar.activation(
                                    sig, pr[i], AF.Sigmoid,
                                    bias=bqk_sb[:, wi:wi + 1])
                                nc.vector.scalar_tensor_tensor(
                                    out=silu[:, isl], in0=pr[i],
                                    scalar=bqk_sb[:, wi:wi + 1], in1=sig,
                                    op0=OP.add, op1=OP.mult)
                            else:
                                nc.scalar.activation(sig, pr[i], AF.Sigmoid)
                                nc.vector.tensor_tensor(
                                    silu[:, isl], pr[i], sig, OP.mult)
                        sq = xw.tile([128, N], F32R, name="sq", tag="sq",
                                     bufs=1)
                        nc.scalar.activation(sq, silu, AF.Square)
                        scl = xw.tile([2, N], F32, name="scl", tag="scl",
                                      bufs=1)
                        for i in range(4):
                            isl = slice(i * 512, (i + 1) * 512)
                            nsq = pp.tile([2, 512], F32, name="nsq",
                                          tag="nsq", bufs=1)
                            nc.tensor.matmul(
                                nsq, lhsT=sel_stats, rhs=sq[:, isl],
                                start=True, stop=True)
                            nc.scalar.activation(scl[:, isl], nsq, AF.Ln)
                        sclr = xw.tile([2, N], F32R, name="sclr", tag="sclr",
                                       bufs=1)
                        nc.scalar.activation(
                            sclr, scl, AF.Exp, scale=-0.5,
                            bias=(lnT_t if wname == "q" else zero2_t))
                        for i in range(4):
                            isl = slice(i * 512, (i + 1) * 512)
                            scb = pp.tile([128, 512], F32, name="scb",
                                          tag="scb", bufs=1)
                            nc.tensor.matmul(
                                scb, lhsT=sel_bcast, rhs=sclr[:, isl],
                                start=True, stop=True)
                            nc.vector.tensor_tensor(
                                dst[:, isl], silu[:, isl], scb, OP.mult)

                    # V projection (token-major, both heads: 512 cols)
                    for tt in range(NT):
                        vpr = pp.tile([128, 512], F32, name="vpr", tag="vpr",
                                      bufs=2)
                        for cc in range(CCN):
                            nc.tensor.matmul(
                                vpr,
                                lhsT=xnT[cc][:, tt * 128:(tt + 1) * 128],
                                rhs=wv_sb[cc],
                                start=(cc == 0), stop=(cc == CCN - 1),
                            )
                        vs = xw.tile([128, 512], F32, name="vs", tag="vs",
                                     bufs=2)
                        if has_vg_bias:
                            vy = xw.tile([128, 512], F32, name="vy", tag="vy",
                                         bufs=2)
                            nc.vector.tensor_tensor(vy, vpr, bv_sb, OP.add)
                            nc.scalar.activation(vs, vy, AF.Sigmoid)
                            nc.vector.tensor_tensor(v_sb[tt], vy, vs, OP.mult)
                        else:
                            nc.scalar.activation(vs, vpr, AF.Sigmoid)
                            nc.vector.tensor_tensor(v_sb[tt], vpr, vs, OP.mult)

                    # gate projection (dv-major)
                    for q in range(4):
                        gpr = [
                            pp.tile([128, 512], F32, name=f"pr{i}",
                                    tag=f"pr{i}", bufs=1)
                            for i in range(4)
                        ]
                        for cc in range(CCN):
                            for i in range(4):
                                nc.tensor.matmul(
                                    gpr[i],
                                    lhsT=wg_sb[cc][:, q * 128:(q + 1) * 128],
                                    rhs=xnT[cc][:, i * 512:(i + 1) * 512],
                                    start=(cc == 0), stop=(cc == CCN - 1),
                                )
                        for i in range(4):
                            isl = slice(i * 512, (i + 1) * 512)
                            gs = xw.tile([128, 512], F32, name="gs",
                                         tag="sig", bufs=2)
                            if has_vg_bias:
                                nc.scalar.activation(
                                    gs, gpr[i], AF.Sigmoid,
                                    bias=bg_sb[:, q:q + 1])
                                nc.vector.scalar_tensor_tensor(
                                    out=gateT[q][:, isl], in0=gpr[i],
                                    scalar=bg_sb[:, q:q + 1], in1=gs,
                                    op0=OP.add, op1=OP.mult)
                            else:
                                nc.scalar.activation(gs, gpr[i], AF.Sigmoid)
                                nc.vector.tensor_tensor(
                                    gateT[q][:, isl], gpr[i], gs, OP.mult)

                # ------------- phase A: attention ------------------------
                with tc.tile_pool(name="at", bufs=1) as at, \
                     tc.tile_pool(name="atps", bufs=1, space="PSUM") as atps:

                    def make_epilogue_e2(h_, isl_, racc_v_, racc_p_):
                        """Row-sum reduce (broadcast to 128 parts) + 1/x.

                        lhsT = all-ones [128,128]: every output partition
                        receives the same partition-axis sum, so the
                        broadcast is free.
                        """
                        def e2():
                            rs_b = atps.tile([128, 1024], F32, name="rs_b",
                                             tag="oa0", bufs=1)
                            for t2 in range(2):
                                fs = slice(t2 * 512, (t2 + 1) * 512)
                                nc.tensor.matmul(
                                    rs_b[:, fs], lhsT=ones_sq,
                                    rhs=racc_v_[:, fs],
                                    start=True, stop=False)
                                nc.tensor.matmul(
                                    rs_b[:, fs], lhsT=ones_sq,
                                    rhs=racc_p_[:, fs],
                                    start=False, stop=True)
                            rlb = at.tile([128, 1024], F32, name="rlb",
                                          tag="rlb", bufs=2)
                            nc.scalar.activation(rlb, rs_b, AF.Ln)
                            rbs = at.tile([128, 1024], F32, name="rbs",
                                          tag="rbs", bufs=2)
                            nc.scalar.activation(rbs, rlb, AF.Exp, scale=-1.0)
                            return rbs
                        return e2

                    pending2 = None  # -> returns rbs
                    pending3 = None  # final out2T muls, needs rbs
                    for h in range(2):
                        hr = slice(h * 64, (h + 1) * 64)
                        for ic in range(2):
                            i0 = ic * 1024
                            isl = slice(i0, i0 + 1024)
                            oa = [
                                atps.tile([128, 1024], F32, name=f"oa{dc}",
                                          tag=f"oa{dc}", bufs=1)
                                for dc in range(2)
                            ]
                            racc_v = at.tile([128, 1024], F32R, name="racc_v",
                                             tag="racc_v", bufs=2)
                            racc_p = at.tile([128, 1024], F32R, name="racc_p",
                                             tag="racc_p", bufs=2)
                            prev_aet = None
                            for j in range(NT):
                                jsl = slice(j * 128, (j + 1) * 128)
                                dts = atps.tile([128, 1024], F32, name="dots",
                                                tag="dots", bufs=2)
                                for t2 in range(2):
                                    fs = slice(t2 * 512, (t2 + 1) * 512)
                                    nc.tensor.matmul(
                                        dts[:, fs], lhsT=kst[hr, jsl],
                                        rhs=qst[hr, i0 + t2 * 512:
                                                i0 + (t2 + 1) * 512],
                                        start=True, stop=True)
                                pb = at.tile([128, 1024], BF16, name="pb",
                                             tag="pb", bufs=4)
                                nc.sync.dma_start(
                                    pb, pbt_d.ap()[h, jsl, i0:i0 + 1024])
                                aer = at.tile([128, 1024], BF16, name="aer",
                                              tag="aer", bufs=3)
                                nc.scalar.activation(aer, dts, AF.Exp)
                                aet = at.tile([128, 1024], BF16, name="aet",
                                              tag="aet", bufs=4)
                                nc.vector.tensor_tensor(aet, aer, pb, OP.mult)
                                if j == 0 and pending2 is not None:
                                    rbs = pending2()
                                    pending2 = None
                                if j == 1 and pending3 is not None:
                                    pending3(rbs)
                                    pending3 = None
                                # pair-sum row accumulation (DVE/Pool split)
                                if j % 2 == 0:
                                    prev_aet = aet
                                else:
                                    pidx = j // 2
                                    eng = nc.vector if pidx % 2 == 0 \
                                        else nc.gpsimd
                                    pair = at.tile([128, 1024], BF16,
                                                   name="pair", tag="pair",
                                                   bufs=2)
                                    eng.tensor_tensor(pair, prev_aet, aet,
                                                      OP.add)
                                    racc = racc_v if pidx % 2 == 0 else racc_p
                                    if pidx < 2:
                                        eng.tensor_copy(racc, pair)
                                    else:
                                        eng.tensor_tensor(racc, racc, pair,
                                                          OP.add)
                                for dc in range(2):
                                    vsl = slice(h * 256 + dc * 128,
                                                h * 256 + (dc + 1) * 128)
                                    for t2 in range(2):
                                        fs = slice(t2 * 512, (t2 + 1) * 512)
                                        nc.tensor.matmul(
                                            oa[dc][:, fs],
                                            lhsT=v_sb[j][:, vsl],
                                            rhs=aet[:, fs],
                                            start=(j == 0), stop=(j == NT - 1))
                            # epilogue part 1: free oa now (og = oa * gate)
                            og = []
                            for dc in range(2):
                                q = h * 2 + dc
                                ogt = at.tile([128, 1024], F32,
                                              name=f"og{dc}", tag=f"og{dc}",
                                              bufs=2)
                                nc.vector.tensor_tensor(
                                    ogt, oa[dc], gateT[q][:, isl], OP.mult)
                                og.append(ogt)
                            pending2 = make_epilogue_e2(h, isl, racc_v, racc_p)

                            def make_e3(h_, isl_, og_):
                                def e3(rbs_):
                                    q0, q1 = h_ * 2, h_ * 2 + 1
                                    nc.gpsimd.tensor_tensor(
                                        out2T[q0][:, isl_], og_[0], rbs_,
                                        OP.mult)
                                    nc.vector.tensor_tensor(
                                        out2T[q1][:, isl_], og_[1], rbs_,
                                        OP.mult)
                                return e3
                            pending3 = make_e3(h, isl, og)
                    # drain last epilogue
                    rbs = pending2()
                    pending3(rbs)

                # ------------- phase W: final Wo contraction -------------
                with tc.tile_pool(name="fo", bufs=1) as fo, \
                     tc.tile_pool(name="fops", bufs=1, space="PSUM") as fops:
                    for it in range(NT):
                        tsl = slice(it * 128, (it + 1) * 128)
                        for co in range(2):
                            fps = fops.tile([128, 512], F32, name="fps",
                                            tag="fps", bufs=4)
                            for q in range(4):
                                nc.tensor.matmul(
                                    fps,
                                    lhsT=out2T[q][:, tsl],
                                    rhs=wo_sb[q][:, co * 512:(co + 1) * 512],
                                    start=(q == 0), stop=(q == 3),
                                )
                            ot = fo.tile([128, 512], F32, name="ot",
                                         tag="ot", bufs=4)
                            if (it * 2 + co) % 2 == 0:
                                nc.scalar.activation(ot, fps, AF.Copy)
                            else:
                                nc.vector.tensor_copy(ot, fps)
                            nc.sync.dma_start(
                                out_ap[tsl, co * 512:(co + 1) * 512], ot)
    if split_waits:
        split_excess_waits(nc)
    return nc


# ---- host side ---------------------------------------------------------
def _sel_stats():
    m = np.zeros((128, 2), np.float32)
    m[0:64, 0] = 1.0
    m[64:128, 1] = 1.0
    return m


def _sel_bcast():
    m = np.zeros((2, 128), np.float32)
    m[0, 0:64] = 1.0
    m[1, 64:128] = 1.0
    return m


def prep_core_inputs(inputs: dict) -> tuple[list[dict], bool, bool]:
    x = np.asarray(inputs["x"], np.float32)
    ln_w = np.asarray(inputs["ln_w"], np.float32)
    ln_b = np.asarray(inputs["ln_b"], np.float32)
    Wvg = np.asarray(inputs["Wvg"], np.float32)
    bvg = np.asarray(inputs["bvg"], np.float32)
    Wqk = np.asarray(inputs["Wqk"], np.float32)
    bqk = np.asarray(inputs["bqk"], np.float32)
    Wo = np.asarray(inputs["Wo"], np.float32)
    pos_bias = np.asarray(inputs["pos_bias"], np.float32)

    has_qk_bias = bool(np.any(bqk != 0.0))
    has_vg_bias = bool(np.any(bvg != 0.0))

    # host layernorm (token-major), then transpose to [C, N] bf16
    mu = x.mean(-1, keepdims=True)
    var = x.var(-1, keepdims=True)
    xn = (x - mu) / np.sqrt(var + EPS) * ln_w + ln_b
    xnT = [np.ascontiguousarray(xn[b].T).astype(ml_dtypes.bfloat16)
           for b in range(B)]

    pbT = np.ascontiguousarray(np.exp(pos_bias.transpose(0, 2, 1))).astype(
        ml_dtypes.bfloat16)

    in_maps = []
    for c in range(8):
        b = c // 4
        h0 = 2 * (c % 4)
        heads = (h0, h0 + 1)
        qcols = [np.arange(h * 128, h * 128 + 64) for h in heads]
        kcols = [np.arange(h * 128 + 64, (h + 1) * 128) for h in heads]
        vcols = [np.arange(h * 256, (h + 1) * 256) for h in heads]
        gcols = [2 * C + np.arange(h * 256, (h + 1) * 256) for h in heads]

        wq = np.ascontiguousarray(
            Wqk[:, np.concatenate(qcols)]).astype(ml_dtypes.bfloat16)
        wk = np.ascontiguousarray(
            Wqk[:, np.concatenate(kcols)]).astype(ml_dtypes.bfloat16)
        wv = np.ascontiguousarray(
            Wvg[:, np.concatenate(vcols)]).astype(ml_dtypes.bfloat16)
        wg = np.ascontiguousarray(
            Wvg[:, np.concatenate(gcols)]).astype(ml_dtypes.bfloat16)
        worows = np.concatenate(
            [np.arange(h * 256, (h + 1) * 256) for h in heads])
        wo = np.ascontiguousarray(Wo[worows, :]).astype(ml_dtypes.bfloat16)

        im = {
            "xnt": xnT[b],
            "wq": wq, "wk": wk, "wv": wv, "wg": wg, "wo": wo,
            "pbt": np.ascontiguousarray(pbT[list(heads)]),
            "sel_stats": _sel_stats(), "sel_bcast": _sel_bcast(),
            "onessq": np.ones((128, 128), np.float32),
        }
        if has_qk_bias:
            bq = bqk[np.concatenate(qcols)]
            bk = bqk[np.concatenate(kcols)]
            im["bqk"] = np.stack([bq, bk], axis=1).astype(np.float32)
        if has_vg_bias:
            bgv = bvg[np.concatenate(gcols)]
            im["bv"] = bvg[np.concatenate(vcols)].astype(np.float32)
            im["bg"] = np.stack([bgv[0:128], bgv[128:256],
                                 bgv[256:384], bgv[384:512]], axis=1
                                ).astype(np.float32)
        in_maps.append(im)
    return in_maps, has_qk_bias, has_vg_bias


_prog_cache: dict = {}


def _get_program(temperature: float, has_qk_bias: bool,
                 has_vg_bias: bool) -> bass.Bass:
    key = (round(float(temperature), 9), has_qk_bias, has_vg_bias)
    if key not in _prog_cache:
        _prog_cache[key] = build_program(
            float(temperature), has_qk_bias, has_vg_bias)
    return _prog_cache[key]


def kernel(**inputs) -> np.ndarray:
    in_maps, has_qk_bias, has_vg_bias = prep_core_inputs(inputs)
    nc = _get_program(float(np.asarray(inputs["temperature"])),
                      has_qk_bias, has_vg_bias)
    res = run_bass_kernel_spmd(nc, in_maps, list(range(8)))
    bo = np.asarray(inputs["bo"], np.float32)
    out = np.zeros((B, N, C), np.float32)
    for c in range(8):
        out[c // 4] += res.results[c]["out"]
    out += bo
    return out
